# revision 17
# baseline (speedup 1.0000x reference)
"""GAU (gated attention unit) Trainium2 Bass kernel.

Computation (per batch item b):
    z = layernorm(x[b]) ; hg = silu(z @ w_hidden + b_hidden) ; v, gate = split(hg)
    qk = silu(z @ w_qk + b_qk) ; q = qk*g0+b0 ; k = qk*g1+b1
    attn = relu(q @ k^T / S)^2 * mask ; out = ((attn @ v) * gate) @ w_out + b_out + x[b]

Sharding: 8 cores = 4 batch items x 2 hidden-halves. Core (b, h) computes the
h-th 768-wide half of the 1536 hidden dim (v/gate columns, w_out rows) for
batch b and returns a partial [2048, 768] output; the host sums the two
partials per batch and adds the residual + b_out.

On-device everything is "feature-major" (features on SBUF partitions) so that
every matmul contracts over the partition dim. All matmul operands are bf16
with fp32 PSUM accumulation; the final output of the block is ~1e5x smaller
than the residual, so bf16 matmul rounding is diluted far below fp32 epsilon
in the returned tensor.

Layernorm mean/var are computed with ones-vector matmuls on the tensor engine
(features live on partitions), the per-token scale r=rsqrt(var+eps) and r*mu
are reshaped to [128,16] through DRAM so the finishing arithmetic runs on all
128 DVE lanes, then broadcast back across partitions with a stride-0 DMA.
"""

import numpy as np
import ml_dtypes

import concourse.bacc as bacc
import concourse.bass as bass
import concourse.mybir as mybir
import concourse.tile as tile
from concourse.bass_utils import run_bass_kernel_spmd

BF16 = mybir.dt.bfloat16
F32 = mybir.dt.float32
AF = mybir.ActivationFunctionType
ALU = mybir.AluOpType

B, S, D, QK, HID = 4, 2048, 768, 128, 1536
H = HID // 2          # per-core hidden half
KT = D // 128         # 6 k-tiles over the 768 feature dim
NCH, CH = 4, 512      # token chunks for the moving operand
JT = S // 128         # 16 token tiles
MT = CH // 128        # 4 output m-tiles per chunk
EPS = 1e-5

_CACHE: dict = {}


def _build(apply_mask: bool, apply_bv: bool):
    nc = bacc.Bacc("TRN2", target_bir_lowering=False, debug=False)

    # Per-core DRAM inputs (host pre-arranges to partition-major layouts).
    xt = nc.dram_tensor("xt", [128, KT, S], BF16, kind="ExternalInput")
    wqk = nc.dram_tensor("wqk", [128, KT, QK], BF16, kind="ExternalInput")
    wv = nc.dram_tensor("wv", [128, KT, H], BF16, kind="ExternalInput")
    wg = nc.dram_tensor("wg", [128, KT, KT, 128], BF16, kind="ExternalInput")
    wo = nc.dram_tensor("wo", [128, KT, D], BF16, kind="ExternalInput")
    bqk = nc.dram_tensor("bqk", [128, 1], F32, kind="ExternalInput")
    bg = nc.dram_tensor("bg", [128, KT], F32, kind="ExternalInput")
    osv = nc.dram_tensor("osv", [128, 4], F32, kind="ExternalInput")
    bv = mk = None
    if apply_bv:
        bv = nc.dram_tensor("bv", [1, H], F32, kind="ExternalInput")
    if apply_mask:
        mk = nc.dram_tensor("mk", [128, JT], F32, kind="ExternalInput")
    out = nc.dram_tensor("out", [S, D], F32, kind="ExternalOutput")

    with tile.TileContext(nc) as tc:
        with (
            tc.tile_pool(name="singles", bufs=1) as singles,
            tc.tile_pool(name="zt", bufs=1) as ztp,
            tc.tile_pool(name="qkp", bufs=1) as qkp,
            tc.tile_pool(name="vp", bufs=1) as vp,
            tc.tile_pool(name="gp", bufs=1) as gp,
            tc.tile_pool(name="ps_stats", bufs=1, space="PSUM") as ps_stats,
            tc.tile_pool(name="ps_mm", bufs=4, space="PSUM") as ps_mm,
            tc.tile_pool(name="ps_sm", bufs=2, space="PSUM") as ps_sm,
            tc.tile_pool(name="dram", bufs=1, space="DRAM") as dram,
        ):
            # ---- constants / weights. xt arrives per-(chunk,k) on the sync
            # queue first so the stats pipeline starts as early as possible;
            # the bulky weights follow on the same queue (they are not needed
            # until the first hg matmuls ~25us in).
            ones_sb = singles.tile([128, 128], BF16, tag="ones", name="ones")
            nc.vector.memset(ones_sb[:], 1.0)
            eps_sb = singles.tile([128, 1], F32, tag="eps", name="eps")
            nc.vector.memset(eps_sb[:], EPS)

            wqk_sb = singles.tile([128, KT, QK], BF16, tag="wqk", name="wqk")
            wv_sb = singles.tile([128, KT, H], BF16, tag="wv", name="wv")
            wg_sb = singles.tile([128, KT, KT, 128], BF16, tag="wg", name="wg")
            wo_sb = singles.tile([128, KT, D], BF16, tag="wo", name="wo")
            bqk_sb = singles.tile([128, 1], F32, tag="bqk", name="bqk")
            bg_sb = singles.tile([128, KT], F32, tag="bg", name="bg")
            osv_sb = singles.tile([128, 4], F32, tag="osv", name="osv")
            bv_sb = None
            if apply_bv:
                bv_sb = singles.tile([128, H], F32, tag="bv", name="bv")
            mk_sb = None
            if apply_mask:
                mk_sb = singles.tile([128, JT], F32, tag="mk", name="mk")

            zt_sb = [ztp.tile([128, S], BF16, tag=f"zt{k}", name=f"zt{k}")
                     for k in range(KT)]
            qt_sb = qkp.tile([128, S], BF16, tag="qt", name="qt")
            kt_sb = qkp.tile([128, S], BF16, tag="kt", name="kt")
            v_sb = [vp.tile([128, H], BF16, tag=f"v{j}", name=f"v{j}")
                    for j in range(JT)]
            gt_sb = [gp.tile([128, S], BF16, tag=f"g{m}", name=f"g{m}")
                     for m in range(KT)]

            # ============ Phase A: layernorm stats on the PE ============
            # The whole stats -> r -> broadcast chain runs PER CHUNK so the
            # DMA/finish latency of chunk c overlaps the PE stats of c+1..
            with (
                tc.tile_pool(name="ln", bufs=1) as ln1,
                tc.tile_pool(name="lnt", bufs=4) as ln3,
                tc.tile_pool(name="lnf", bufs=2) as lnf,
            ):
                xt_sb = ln1.tile([128, KT, S], BF16, tag="xt", name="xt")
                for c in range(NCH):
                    for k in range(KT):
                        cs = bass.ts(c, CH)
                        nc.sync.dma_start(xt_sb[:, k, cs], xt[:, k, cs])
                nc.sync.dma_start(wqk_sb[:], wqk[:])
                nc.sync.dma_start(wv_sb[:], wv[:])
                nc.scalar.dma_start(wg_sb[:], wg[:])
                nc.scalar.dma_start(wo_sb[:], wo[:])
                nc.scalar.dma_start(bqk_sb[:], bqk[:])
                nc.scalar.dma_start(bg_sb[:], bg[:])
                nc.scalar.dma_start(osv_sb[:], osv[:])
                if apply_bv:
                    nc.scalar.dma_start(
                        out=bv_sb[:],
                        in_=bass.AP(tensor=bv.tensor, offset=bv.offset,
                                    ap=[[0, 128], [1, H]]))
                if apply_mask:
                    nc.scalar.dma_start(mk_sb[:], mk[:])

                FWc = CH // 128
                rr_bcs = [None] * NCH

                # ===== Phases A-D interleaved per token chunk:
                # stats -> r -> zT -> qk -> v -> gate.  While chunk c's
                # stats/finish DMA chain is in flight the PE runs chunk c+1
                # stats, so it never stalls on the layernorm scalars.
                for c in range(NCH):
                    cs = bass.ts(c, CH)
                    ps_sum = ps_stats.tile([128, CH], F32, tag="pssum", name="pssum")
                    ps_ssq = ps_stats.tile([128, CH], F32, tag="psssq", name="psssq")
                    for k in range(KT):
                        xsq = ln3.tile([128, CH], BF16, tag="xsq", name="xsq")
                        nc.vector.tensor_mul(xsq[:], xt_sb[:, k, cs], xt_sb[:, k, cs])
                        nc.tensor.matmul(ps_sum[:], ones_sb[:], xt_sb[:, k, cs],
                                         start=(k == 0), stop=(k == KT - 1))
                        nc.tensor.matmul(ps_ssq[:], ones_sb[:], xsq[:],
                                         start=(k == 0), stop=(k == KT - 1))
                    # ones is a [128,128] matrix, so ps_sum/ps_ssq hold the
                    # per-token sums already broadcast to every partition —
                    # finish r = rsqrt(var+eps) and r*mu full-lane, on chip.
                    mu_bc = lnf.tile([128, CH], F32, tag="mu_bc", name="mu_bc")
                    nc.vector.tensor_scalar_mul(mu_bc[:], ps_sum[:], 1.0 / D)
                    musq = lnf.tile([128, CH], F32, tag="musq", name="musq")
                    nc.vector.tensor_mul(musq[:], mu_bc[:], mu_bc[:])
                    # var = E[x^2] - mu^2 = ssq/D - musq
                    var_bc = lnf.tile([128, CH], F32, tag="var_bc", name="var_bc")
                    nc.vector.scalar_tensor_tensor(
                        var_bc[:], ps_ssq[:], 1.0 / D, musq[:],
                        op0=ALU.mult, op1=ALU.subtract)
                    std_bc = lnf.tile([128, CH], F32, tag="std_bc", name="std_bc")
                    nc.scalar.activation(std_bc[:], var_bc[:], AF.Sqrt, bias=eps_sb[:])
                    rr_bc = ln1.tile([128, 2, CH], F32, tag=f"rrbc{c}",
                                     name=f"rrbc{c}")
                    nc.vector.reciprocal(rr_bc[:, 0, :], std_bc[:])
                    nc.vector.tensor_mul(rr_bc[:, 1, :], mu_bc[:], rr_bc[:, 0, :])
                    rr_bcs[c] = rr_bc

                    r_bc = rr_bc[:, 0, :]
                    rmu_bc = rr_bc[:, 1, :]
                    # zT = xT * r - r*mu  (bf16 throughout for 2x DVE mode)
                    for k in range(KT):
                        t = ln3.tile([128, CH], BF16, tag="lnt", name="lnt")
                        nc.vector.tensor_mul(t[:], xt_sb[:, k, cs], r_bc[:])
                        nc.vector.tensor_sub(zt_sb[k][:, cs], t[:], rmu_bc[:])
                    # qk branch
                    ps = ps_mm.tile([128, CH], F32, tag="mm", name="mm")
                    for k in range(KT):
                        nc.tensor.matmul(ps[:], wqk_sb[:, k, :], zt_sb[k][:, cs],
                                         start=(k == 0), stop=(k == KT - 1))
                    sl = ln3.tile([128, CH], F32, tag="qksilu", name="qksilu")
                    nc.scalar.activation(sl[:], ps[:], AF.Silu, bias=bqk_sb[:])
                    nc.vector.tensor_scalar(qt_sb[:, cs], sl[:],
                                            osv_sb[:, 0:1], osv_sb[:, 1:2],
                                            op0=ALU.mult, op1=ALU.add)
                    nc.vector.tensor_scalar(kt_sb[:, cs], sl[:],
                                            osv_sb[:, 2:3], osv_sb[:, 3:4],
                                            op0=ALU.mult, op1=ALU.add)
                    # v tiles for this chunk's tokens (token-major)
                    for j in range(c * MT, (c + 1) * MT):
                        js = bass.ts(j, 128)
                        ps5 = ps_mm.tile([128, CH], F32, tag="mm", name="mm")
                        ps2 = ps_sm.tile([128, H - CH], F32, tag="sm", name="sm")
                        for k in range(KT):
                            nc.tensor.matmul(ps5[:], zt_sb[k][:, js],
                                             wv_sb[:, k, 0:CH],
                                             start=(k == 0), stop=(k == KT - 1))
                            nc.tensor.matmul(ps2[:], zt_sb[k][:, js],
                                             wv_sb[:, k, CH:H],
                                             start=(k == 0), stop=(k == KT - 1))
                        if apply_bv:
                            t5 = ln3.tile([128, CH], F32, tag="bvt5", name="bvt5")
                            t2 = ln3.tile([128, H - CH], F32, tag="bvt2", name="bvt2")
                            nc.vector.tensor_add(t5[:], ps5[:], bv_sb[:, 0:CH])
                            nc.vector.tensor_add(t2[:], ps2[:], bv_sb[:, CH:H])
                            nc.scalar.activation(v_sb[j][:, 0:CH], t5[:], AF.Silu)
                            nc.scalar.activation(v_sb[j][:, CH:H], t2[:], AF.Silu)
                        else:
                            nc.scalar.activation(v_sb[j][:, 0:CH], ps5[:], AF.Silu)
                            nc.scalar.activation(v_sb[j][:, CH:H], ps2[:], AF.Silu)
                    # gateT tiles for this chunk (feature-major)
                    for m in range(KT):
                        psg = ps_mm.tile([128, CH], F32, tag="mm", name="mm")
                        for k in range(KT):
                            nc.tensor.matmul(psg[:], wg_sb[:, k, m, :],
                                             zt_sb[k][:, cs],
                                             start=(k == 0), stop=(k == KT - 1))
                        nc.scalar.activation(gt_sb[m][:, cs], psg[:], AF.Silu,
                                             bias=bg_sb[:, m:m + 1])

            # ================= Phase E: attention + output ================
            with (
                tc.tile_pool(name="attn", bufs=3) as attnp,
                tc.tile_pool(name="tmp", bufs=4) as tmp,
                tc.tile_pool(name="outp", bufs=2) as outp,
            ):
                for c in range(NCH):
                    cs = bass.ts(c, CH)
                    at = [attnp.tile([128, CH], BF16, tag=f"at{j}", name=f"at{j}")
                          for j in range(JT)]
                    for j in range(JT):
                        ps = ps_mm.tile([128, CH], F32, tag="mm", name="mm")
                        nc.tensor.matmul(ps[:], kt_sb[:, bass.ts(j, 128)],
                                         qt_sb[:, cs], start=True, stop=True)
                        rl = tmp.tile([128, CH], F32, tag="relu", name="relu")
                        if j % 2 == 0:
                            nc.scalar.activation(rl[:], ps[:], AF.Relu)
                        else:
                            nc.vector.tensor_relu(rl[:], ps[:])
                        nc.vector.tensor_mul(at[j][:], rl[:], rl[:])
                        if apply_mask:
                            nc.vector.tensor_scalar_mul(at[j][:], at[j][:],
                                                        mk_sb[:, j:j + 1])
                    gd = [attnp.tile([128, CH], BF16, tag=f"gd{d}", name=f"gd{d}")
                          for d in range(KT)]
                    for d in range(KT):
                        ds_ = bass.ts(d, 128)
                        ps = ps_mm.tile([128, CH], F32, tag="mm", name="mm")
                        for j in range(JT):
                            nc.tensor.matmul(ps[:], v_sb[j][:, ds_], at[j][:],
                                             start=(j == 0), stop=(j == JT - 1))
                        nc.vector.tensor_mul(gd[d][:], ps[:], gt_sb[d][:, cs])
                    for m in range(MT):
                        ms = bass.ts(m, 128)
                        psa = ps_mm.tile([128, CH], F32, tag="mm", name="mm")
                        psb = ps_sm.tile([128, D - CH], F32, tag="sm", name="sm")
                        for d in range(KT):
                            nc.tensor.matmul(psa[:], gd[d][:, ms],
                                             wo_sb[:, d, 0:CH],
                                             start=(d == 0), stop=(d == KT - 1))
                            nc.tensor.matmul(psb[:], gd[d][:, ms],
                                             wo_sb[:, d, CH:D],
                                             start=(d == 0), stop=(d == KT - 1))
                        ot = outp.tile([128, D], F32, tag="ot", name="ot")
                        nc.vector.tensor_copy(ot[:, 0:CH], psa[:])
                        nc.vector.tensor_copy(ot[:, CH:D], psb[:])
                        nc.sync.dma_start(
                            out[c * CH + m * 128:c * CH + (m + 1) * 128, :], ot[:])

    nc.compile()
    return nc


def _tile_pm(w, kt):
    """[kt*128, n] -> [128, kt, n] partition-major, contiguous."""
    n = w.shape[1]
    return np.ascontiguousarray(
        w.reshape(kt, 128, n).transpose(1, 0, 2)).astype(ml_dtypes.bfloat16)


def _prepare(inputs):
    x = np.asarray(inputs["hidden_states"], np.float32)
    mask = np.asarray(inputs["attention_mask"])
    ln_g = np.asarray(inputs["ln_gamma"], np.float32)
    ln_b = np.asarray(inputs["ln_beta"], np.float32)
    w_h = np.asarray(inputs["w_hidden"], np.float32)
    b_h = np.asarray(inputs["b_hidden"], np.float32)
    w_qk = np.asarray(inputs["w_qk"], np.float32)
    b_qk = np.asarray(inputs["b_qk"], np.float32)
    os_g = np.asarray(inputs["os_gamma"], np.float32)
    os_b = np.asarray(inputs["os_beta"], np.float32)
    w_o = np.asarray(inputs["w_out"], np.float32)
    b_o = np.asarray(inputs["b_out"], np.float32)

    # fold layernorm affine into the weights that consume normed activations
    wh_f = ln_g[:, None] * w_h
    bh_f = b_h + ln_b @ w_h
    wqk_f = ln_g[:, None] * w_qk
    bqk_f = b_qk + ln_b @ w_qk

    apply_mask = not bool(mask.all())
    apply_bv = bool(np.any(bh_f[:HID]))

    osv = np.stack([os_g[0] / S, os_b[0] / S, os_g[1], os_b[1]], axis=1)
    osv = np.ascontiguousarray(osv).astype(np.float32)  # [128, 4]

    per_h = []
    for h in range(2):
        w_v = wh_f[:, h * H:(h + 1) * H]
        w_g = wh_f[:, HID + h * H:HID + (h + 1) * H]
        b_v = bh_f[h * H:(h + 1) * H]
        b_g = bh_f[HID + h * H:HID + (h + 1) * H]
        m = {
            "wqk": _tile_pm(wqk_f, KT),
            "wv": _tile_pm(w_v, KT),
            "wg": np.ascontiguousarray(
                w_g.reshape(KT, 128, KT, 128).transpose(1, 0, 2, 3)
            ).astype(ml_dtypes.bfloat16),
            "wo": _tile_pm(w_o[h * H:(h + 1) * H, :], KT),
            "bqk": np.ascontiguousarray(bqk_f.reshape(128, 1)),
            "bg": np.ascontiguousarray(b_g.reshape(KT, 128).T),
            "osv": osv,
        }
        if apply_bv:
            m["bv"] = np.ascontiguousarray(b_v.reshape(1, H))
        per_h.append(m)

    xts = []
    mks = []
    for b in range(B):
        xts.append(np.ascontiguousarray(
            x[b].T.reshape(KT, 128, S).transpose(1, 0, 2)
        ).astype(ml_dtypes.bfloat16))
        if apply_mask:
            mks.append(np.ascontiguousarray(
                mask[b].astype(np.float32).reshape(JT, 128).T))

    in_maps = []
    for c in range(8):
        b, h = c // 2, c % 2
        m = dict(per_h[h])
        m["xt"] = xts[b]
        if apply_mask:
            m["mk"] = mks[b]
        in_maps.append(m)
    return in_maps, apply_mask, apply_bv, x, b_o


def _run(inputs, **run_kwargs):
    in_maps, apply_mask, apply_bv, x, b_o = _prepare(inputs)
    key = (apply_mask, apply_bv)
    if key not in _CACHE:
        _CACHE[key] = _build(*key)
    nc = _CACHE[key]
    res = run_bass_kernel_spmd(nc, in_maps, core_ids=list(range(8)), **run_kwargs)
    outs = [r["out"] for r in res.results]
    final = np.empty((B, S, D), np.float32)
    for b in range(B):
        final[b] = outs[2 * b] + outs[2 * b + 1] + x[b] + b_o
    return final, res


def kernel(**inputs) -> np.ndarray:
    final, _ = _run(inputs)
    return final


# revision 18
# speedup vs baseline: 1.0278x; 1.0278x over previous
"""GAU (gated attention unit) Trainium2 Bass kernel.

Computation (per batch item b):
    z = layernorm(x[b]) ; hg = silu(z @ w_hidden + b_hidden) ; v, gate = split(hg)
    qk = silu(z @ w_qk + b_qk) ; q = qk*g0+b0 ; k = qk*g1+b1
    attn = relu(q @ k^T / S)^2 * mask ; out = ((attn @ v) * gate) @ w_out + b_out + x[b]

Sharding: 8 cores = 4 batch items x 2 hidden-halves. Core (b, h) computes the
h-th 768-wide half of the 1536 hidden dim (v/gate columns, w_out rows) for
batch b and returns a partial [2048, 768] output; the host sums the two
partials per batch and adds the residual + b_out.

On-device everything is "feature-major" (features on SBUF partitions) so that
every matmul contracts over the partition dim. All matmul operands are bf16
with fp32 PSUM accumulation; the final output of the block is ~1e5x smaller
than the residual, so bf16 matmul rounding is diluted far below fp32 epsilon
in the returned tensor.

Layernorm mean/var are computed with ones-vector matmuls on the tensor engine
(features live on partitions), the per-token scale r=rsqrt(var+eps) and r*mu
are reshaped to [128,16] through DRAM so the finishing arithmetic runs on all
128 DVE lanes, then broadcast back across partitions with a stride-0 DMA.
"""

import numpy as np
import ml_dtypes

import concourse.bacc as bacc
import concourse.bass as bass
import concourse.mybir as mybir
import concourse.tile as tile
from concourse.bass_utils import run_bass_kernel_spmd

BF16 = mybir.dt.bfloat16
F32 = mybir.dt.float32
AF = mybir.ActivationFunctionType
ALU = mybir.AluOpType

B, S, D, QK, HID = 4, 2048, 768, 128, 1536
H = HID // 2          # per-core hidden half
KT = D // 128         # 6 k-tiles over the 768 feature dim
NCH, CH = 4, 512      # token chunks for the moving operand
JT = S // 128         # 16 token tiles
MT = CH // 128        # 4 output m-tiles per chunk
EPS = 1e-5

_CACHE: dict = {}


def _build(apply_mask: bool, apply_bv: bool):
    nc = bacc.Bacc("TRN2", target_bir_lowering=False, debug=False)

    # Per-core DRAM inputs (host pre-arranges to partition-major layouts).
    xt = nc.dram_tensor("xt", [128, KT, S], BF16, kind="ExternalInput")
    wqk = nc.dram_tensor("wqk", [128, KT, QK], BF16, kind="ExternalInput")
    wv = nc.dram_tensor("wv", [128, KT, H], BF16, kind="ExternalInput")
    wg = nc.dram_tensor("wg", [128, KT, KT, 128], BF16, kind="ExternalInput")
    wo = nc.dram_tensor("wo", [128, KT, D], BF16, kind="ExternalInput")
    bqk = nc.dram_tensor("bqk", [128, 1], F32, kind="ExternalInput")
    bg = nc.dram_tensor("bg", [128, KT], F32, kind="ExternalInput")
    osv = nc.dram_tensor("osv", [128, 4], F32, kind="ExternalInput")
    bv = mk = None
    if apply_bv:
        bv = nc.dram_tensor("bv", [1, H], F32, kind="ExternalInput")
    if apply_mask:
        mk = nc.dram_tensor("mk", [128, JT], F32, kind="ExternalInput")
    out = nc.dram_tensor("out", [S, D], F32, kind="ExternalOutput")

    with tile.TileContext(nc) as tc:
        with (
            tc.tile_pool(name="singles", bufs=1) as singles,
            tc.tile_pool(name="zt", bufs=1) as ztp,
            tc.tile_pool(name="qkp", bufs=1) as qkp,
            tc.tile_pool(name="vp", bufs=1) as vp,
            tc.tile_pool(name="gp", bufs=1) as gp,
            tc.tile_pool(name="attn", bufs=2) as attnp,
            tc.tile_pool(name="gdp", bufs=1) as gdp,
            tc.tile_pool(name="tmp", bufs=2) as tmp,
            tc.tile_pool(name="outp", bufs=2) as outp,
            tc.tile_pool(name="ps_stats", bufs=1, space="PSUM") as ps_stats,
            tc.tile_pool(name="ps_mm", bufs=4, space="PSUM") as ps_mm,
            tc.tile_pool(name="ps_sm", bufs=2, space="PSUM") as ps_sm,
        ):
            # ones is a [128,128] matrix: the layernorm-stats matmuls produce
            # per-token sums already broadcast to every output partition.
            ones_sb = singles.tile([128, 128], BF16, tag="ones", name="ones")
            nc.vector.memset(ones_sb[:], 1.0)
            eps_sb = singles.tile([128, 1], F32, tag="eps", name="eps")
            nc.vector.memset(eps_sb[:], EPS)

            wqk_sb = singles.tile([128, KT, QK], BF16, tag="wqk", name="wqk")
            wv_sb = singles.tile([128, KT, H], BF16, tag="wv", name="wv")
            wg_sb = singles.tile([128, KT, KT, 128], BF16, tag="wg", name="wg")
            wo_sb = singles.tile([128, KT, D], BF16, tag="wo", name="wo")
            bqk_sb = singles.tile([128, 1], F32, tag="bqk", name="bqk")
            bg_sb = singles.tile([128, KT], F32, tag="bg", name="bg")
            osv_sb = singles.tile([128, 4], F32, tag="osv", name="osv")
            bv_sb = None
            if apply_bv:
                bv_sb = singles.tile([128, H], F32, tag="bv", name="bv")
            mk_sb = None
            if apply_mask:
                mk_sb = singles.tile([128, JT], F32, tag="mk", name="mk")

            zt_sb = [ztp.tile([128, S], BF16, tag=f"zt{k}", name=f"zt{k}")
                     for k in range(KT)]
            qt_sb = qkp.tile([128, S], BF16, tag="qt", name="qt")
            kt_sb = qkp.tile([128, S], BF16, tag="kt", name="kt")
            v_sb = [vp.tile([128, H], BF16, tag=f"v{j}", name=f"v{j}")
                    for j in range(JT)]
            gt_sb = [gp.tile([128, S], BF16, tag=f"g{m}", name=f"g{m}")
                     for m in range(KT)]

            # ============ Phases A-D, interleaved per token chunk ==========
            # stats -> r -> zT -> qk -> v -> gate.  While chunk c's finish
            # chain runs on DVE/ACT, the PE continues with chunk c+1 stats.
            with (
                tc.tile_pool(name="xtp", bufs=2) as xtp,
                tc.tile_pool(name="lnt", bufs=4) as ln3,
                tc.tile_pool(name="lnf", bufs=1) as lnf,
                tc.tile_pool(name="rrp", bufs=1) as rrp,
            ):
                # xt streams in per (chunk, k) on the sync queue; weights
                # follow (wqk/wv on sync, the rest on the scalar queue).
                xt_t: dict = {}
                for c in range(NCH):
                    for k in range(KT):
                        cs = bass.ts(c, CH)
                        xtt = xtp.tile([128, CH], BF16, tag=f"xt{k}",
                                       name=f"xt{k}_{c}")
                        nc.sync.dma_start(xtt[:], xt[:, k, cs])
                        xt_t[c, k] = xtt
                nc.sync.dma_start(wqk_sb[:], wqk[:])
                nc.sync.dma_start(wv_sb[:], wv[:])
                nc.scalar.dma_start(wg_sb[:], wg[:])
                nc.scalar.dma_start(wo_sb[:], wo[:])
                nc.scalar.dma_start(bqk_sb[:], bqk[:])
                nc.scalar.dma_start(bg_sb[:], bg[:])
                nc.scalar.dma_start(osv_sb[:], osv[:])
                if apply_bv:
                    nc.scalar.dma_start(
                        out=bv_sb[:],
                        in_=bass.AP(tensor=bv.tensor, offset=bv.offset,
                                    ap=[[0, 128], [1, H]]))
                if apply_mask:
                    nc.scalar.dma_start(mk_sb[:], mk[:])

                for c in range(NCH):
                    cs = bass.ts(c, CH)
                    ps_sum = ps_stats.tile([128, CH], F32, tag="pssum",
                                           name="pssum")
                    ps_ssq = ps_stats.tile([128, CH], F32, tag="psssq",
                                           name="psssq")
                    for k in range(KT):
                        xsq = ln3.tile([128, CH], BF16, tag="xsq", name="xsq")
                        nc.vector.tensor_mul(xsq[:], xt_t[c, k][:], xt_t[c, k][:])
                        nc.tensor.matmul(ps_sum[:], ones_sb[:], xt_t[c, k][:],
                                         start=(k == 0), stop=(k == KT - 1))
                        nc.tensor.matmul(ps_ssq[:], ones_sb[:], xsq[:],
                                         start=(k == 0), stop=(k == KT - 1))
                    # finish r = rsqrt(var+eps), rmu = mu*r — full-lane, on chip
                    mu_bc = lnf.tile([128, CH], F32, tag="mu_bc", name="mu_bc")
                    nc.vector.tensor_scalar_mul(mu_bc[:], ps_sum[:], 1.0 / D)
                    musq = lnf.tile([128, CH], F32, tag="musq", name="musq")
                    nc.vector.tensor_mul(musq[:], mu_bc[:], mu_bc[:])
                    var_bc = lnf.tile([128, CH], F32, tag="var_bc", name="var_bc")
                    nc.vector.scalar_tensor_tensor(
                        var_bc[:], ps_ssq[:], 1.0 / D, musq[:],
                        op0=ALU.mult, op1=ALU.subtract)
                    rr_bc = rrp.tile([128, 2, CH], BF16, tag=f"rrbc{c}",
                                     name=f"rrbc{c}")
                    nc.scalar.activation(rr_bc[:, 0, :], var_bc[:],
                                         AF.Abs_reciprocal_sqrt, bias=eps_sb[:])
                    nc.vector.tensor_mul(rr_bc[:, 1, :], mu_bc[:], rr_bc[:, 0, :])

                    r_bc = rr_bc[:, 0, :]
                    rmu_bc = rr_bc[:, 1, :]
                    # zT = xT * r - r*mu  (bf16 throughout for 2x DVE mode)
                    for k in range(KT):
                        t = ln3.tile([128, CH], BF16, tag="lnt", name="lnt")
                        nc.vector.tensor_mul(t[:], xt_t[c, k][:], r_bc[:])
                        nc.vector.tensor_sub(zt_sb[k][:, cs], t[:], rmu_bc[:])
                    # qk branch
                    ps = ps_mm.tile([128, CH], F32, tag="mm", name="mm")
                    for k in range(KT):
                        nc.tensor.matmul(ps[:], wqk_sb[:, k, :], zt_sb[k][:, cs],
                                         start=(k == 0), stop=(k == KT - 1))
                    sl = ln3.tile([128, CH], F32, tag="qksilu", name="qksilu")
                    nc.scalar.activation(sl[:], ps[:], AF.Silu, bias=bqk_sb[:])
                    nc.vector.tensor_scalar(qt_sb[:, cs], sl[:],
                                            osv_sb[:, 0:1], osv_sb[:, 1:2],
                                            op0=ALU.mult, op1=ALU.add)
                    nc.vector.tensor_scalar(kt_sb[:, cs], sl[:],
                                            osv_sb[:, 2:3], osv_sb[:, 3:4],
                                            op0=ALU.mult, op1=ALU.add)
                    # v tiles for this chunk's tokens (token-major)
                    for j in range(c * MT, (c + 1) * MT):
                        js = bass.ts(j, 128)
                        ps5 = ps_mm.tile([128, CH], F32, tag="mm", name="mm")
                        ps2 = ps_sm.tile([128, H - CH], F32, tag="sm", name="sm")
                        for k in range(KT):
                            nc.tensor.matmul(ps5[:], zt_sb[k][:, js],
                                             wv_sb[:, k, 0:CH],
                                             start=(k == 0), stop=(k == KT - 1))
                            nc.tensor.matmul(ps2[:], zt_sb[k][:, js],
                                             wv_sb[:, k, CH:H],
                                             start=(k == 0), stop=(k == KT - 1))
                        if apply_bv:
                            t5 = ln3.tile([128, CH], F32, tag="bvt5", name="bvt5")
                            t2 = ln3.tile([128, H - CH], F32, tag="bvt2",
                                          name="bvt2")
                            nc.vector.tensor_add(t5[:], ps5[:], bv_sb[:, 0:CH])
                            nc.vector.tensor_add(t2[:], ps2[:], bv_sb[:, CH:H])
                            nc.scalar.activation(v_sb[j][:, 0:CH], t5[:], AF.Silu)
                            nc.scalar.activation(v_sb[j][:, CH:H], t2[:], AF.Silu)
                        else:
                            nc.scalar.activation(v_sb[j][:, 0:CH], ps5[:], AF.Silu)
                            nc.scalar.activation(v_sb[j][:, CH:H], ps2[:], AF.Silu)
                    # gateT tiles for this chunk (feature-major)
                    for m in range(KT):
                        psg = ps_mm.tile([128, CH], F32, tag="mm", name="mm")
                        for k in range(KT):
                            nc.tensor.matmul(psg[:], wg_sb[:, k, m, :],
                                             zt_sb[k][:, cs],
                                             start=(k == 0), stop=(k == KT - 1))
                        nc.scalar.activation(gt_sb[m][:, cs], psg[:], AF.Silu,
                                             bias=bg_sb[:, m:m + 1])

            # ================= Phase E: attention + output ================
            for c in range(NCH):
                cs = bass.ts(c, CH)
                at = [attnp.tile([128, CH], BF16, tag=f"at{j}", name=f"at{j}")
                      for j in range(JT)]
                for j in range(JT):
                    ps = ps_mm.tile([128, CH], F32, tag="mm", name="mm")
                    nc.tensor.matmul(ps[:], kt_sb[:, bass.ts(j, 128)],
                                     qt_sb[:, cs], start=True, stop=True)
                    rl = tmp.tile([128, CH], F32, tag="relu", name="relu")
                    if j % 2 == 0:
                        nc.scalar.activation(rl[:], ps[:], AF.Relu)
                    else:
                        nc.vector.tensor_relu(rl[:], ps[:])
                    nc.vector.tensor_mul(at[j][:], rl[:], rl[:])
                    if apply_mask:
                        nc.vector.tensor_scalar_mul(at[j][:], at[j][:],
                                                    mk_sb[:, j:j + 1])
                gd = [gdp.tile([128, CH], BF16, tag=f"gd{d}", name=f"gd{d}")
                      for d in range(KT)]
                for d in range(KT):
                    ds_ = bass.ts(d, 128)
                    ps = ps_mm.tile([128, CH], F32, tag="mm", name="mm")
                    for j in range(JT):
                        nc.tensor.matmul(ps[:], v_sb[j][:, ds_], at[j][:],
                                         start=(j == 0), stop=(j == JT - 1))
                    nc.vector.tensor_mul(gd[d][:], ps[:], gt_sb[d][:, cs])
                for m in range(MT):
                    ms = bass.ts(m, 128)
                    psa = ps_mm.tile([128, CH], F32, tag="mm", name="mm")
                    psb = ps_sm.tile([128, D - CH], F32, tag="sm", name="sm")
                    for d in range(KT):
                        nc.tensor.matmul(psa[:], gd[d][:, ms],
                                         wo_sb[:, d, 0:CH],
                                         start=(d == 0), stop=(d == KT - 1))
                        nc.tensor.matmul(psb[:], gd[d][:, ms],
                                         wo_sb[:, d, CH:D],
                                         start=(d == 0), stop=(d == KT - 1))
                    ot = outp.tile([128, D], F32, tag="ot", name="ot")
                    nc.vector.tensor_copy(ot[:, 0:CH], psa[:])
                    nc.vector.tensor_copy(ot[:, CH:D], psb[:])
                    nc.sync.dma_start(
                        out[c * CH + m * 128:c * CH + (m + 1) * 128, :], ot[:])

    nc.compile()
    return nc


def _tile_pm(w, kt):
    """[kt*128, n] -> [128, kt, n] partition-major, contiguous."""
    n = w.shape[1]
    return np.ascontiguousarray(
        w.reshape(kt, 128, n).transpose(1, 0, 2)).astype(ml_dtypes.bfloat16)


def _prepare(inputs):
    x = np.asarray(inputs["hidden_states"], np.float32)
    mask = np.asarray(inputs["attention_mask"])
    ln_g = np.asarray(inputs["ln_gamma"], np.float32)
    ln_b = np.asarray(inputs["ln_beta"], np.float32)
    w_h = np.asarray(inputs["w_hidden"], np.float32)
    b_h = np.asarray(inputs["b_hidden"], np.float32)
    w_qk = np.asarray(inputs["w_qk"], np.float32)
    b_qk = np.asarray(inputs["b_qk"], np.float32)
    os_g = np.asarray(inputs["os_gamma"], np.float32)
    os_b = np.asarray(inputs["os_beta"], np.float32)
    w_o = np.asarray(inputs["w_out"], np.float32)
    b_o = np.asarray(inputs["b_out"], np.float32)

    # fold layernorm affine into the weights that consume normed activations
    wh_f = ln_g[:, None] * w_h
    bh_f = b_h + ln_b @ w_h
    wqk_f = ln_g[:, None] * w_qk
    bqk_f = b_qk + ln_b @ w_qk

    apply_mask = not bool(mask.all())
    apply_bv = bool(np.any(bh_f[:HID]))

    osv = np.stack([os_g[0] / S, os_b[0] / S, os_g[1], os_b[1]], axis=1)
    osv = np.ascontiguousarray(osv).astype(np.float32)  # [128, 4]

    per_h = []
    for h in range(2):
        w_v = wh_f[:, h * H:(h + 1) * H]
        w_g = wh_f[:, HID + h * H:HID + (h + 1) * H]
        b_v = bh_f[h * H:(h + 1) * H]
        b_g = bh_f[HID + h * H:HID + (h + 1) * H]
        m = {
            "wqk": _tile_pm(wqk_f, KT),
            "wv": _tile_pm(w_v, KT),
            "wg": np.ascontiguousarray(
                w_g.reshape(KT, 128, KT, 128).transpose(1, 0, 2, 3)
            ).astype(ml_dtypes.bfloat16),
            "wo": _tile_pm(w_o[h * H:(h + 1) * H, :], KT),
            "bqk": np.ascontiguousarray(bqk_f.reshape(128, 1)),
            "bg": np.ascontiguousarray(b_g.reshape(KT, 128).T),
            "osv": osv,
        }
        if apply_bv:
            m["bv"] = np.ascontiguousarray(b_v.reshape(1, H))
        per_h.append(m)

    xts = []
    mks = []
    for b in range(B):
        xts.append(np.ascontiguousarray(
            x[b].T.reshape(KT, 128, S).transpose(1, 0, 2)
        ).astype(ml_dtypes.bfloat16))
        if apply_mask:
            mks.append(np.ascontiguousarray(
                mask[b].astype(np.float32).reshape(JT, 128).T))

    in_maps = []
    for c in range(8):
        b, h = c // 2, c % 2
        m = dict(per_h[h])
        m["xt"] = xts[b]
        if apply_mask:
            m["mk"] = mks[b]
        in_maps.append(m)
    return in_maps, apply_mask, apply_bv, x, b_o


def _run(inputs, **run_kwargs):
    in_maps, apply_mask, apply_bv, x, b_o = _prepare(inputs)
    key = (apply_mask, apply_bv)
    if key not in _CACHE:
        _CACHE[key] = _build(*key)
    nc = _CACHE[key]
    res = run_bass_kernel_spmd(nc, in_maps, core_ids=list(range(8)), **run_kwargs)
    outs = [r["out"] for r in res.results]
    final = np.empty((B, S, D), np.float32)
    for b in range(B):
        final[b] = outs[2 * b] + outs[2 * b + 1] + x[b] + b_o
    return final, res


def kernel(**inputs) -> np.ndarray:
    final, _ = _run(inputs)
    return final
